# revision 23
# baseline (speedup 1.0000x reference)
"""Trainium2 Bass kernel for nn_CAM (channel attention module).

Reference (per batch b):
    f = x[b].reshape(N, C)                      # N = H*W = 4096, C = 512
    G = f^T f                                   # (C, C) channel gram
    A = softmax(G, axis=-1)
    out[b] = gamma * (f @ A) + x[b]

Algebraic folds:
  * out = f (.) s  +  f @ (gamma * R),  where s_d = 1 + gamma * A[d,d] and
    R = A - diag(A).  The gram's diagonal is ~N >> off-diag ~sqrt(N), so
    softmax concentrates on the diagonal; the dominant diagonal term is
    computed exactly on the elementwise path (bf16 f), and only the tiny
    off-diagonal remainder goes through the second matmul, which therefore
    runs in fp8 DoubleRow (2x bf16 PE throughput) with no accuracy cost.
    A[d,d] = 1/esum_d exactly, since exp(G_dd - rowmax) = exp(0) = 1.
  * symmetry: G == G^T, so only upper-triangular 128-blocks are computed
    (free dims 512/384/256/128); the 6 lower blocks are PE-transposed back.

Engine economy (the binding constraint once MM2 is fp8):
  * x is cast-LOADED TWICE by the SWDGE (fp32->bf16 and fp32->fp8), so no
    on-chip cast instructions exist at all; HBM has ~2.6x headroom.
  * Loads/stores are batched 4 chunks / 2 groups per DMA so the Pool/SP
    sequencers spend ~12us, not ~25us, issuing DMAs.
  * GPSIMD (Pool) executes NO tensor ops (hardware runs them far below the
    cost model's estimate); it only triggers DMAs + partition_broadcast.
  * fsk = f (.) s is computed in place over the bf16 chunk (all-SBUF
    all-bf16: DVE 2x/4x modes); the MM2 drain is an ACT/DVE plain PSUM
    copy and the residual add is a wide all-bf16 DVE tensor_tensor.

Layout: n rows are interleaved 2-per-partition (row 256k + 2p + j lives on
partition p, slice j of chunk k): 4KB load / 2KB store descriptors.  The
gram is invariant to the n-permutation; ft/MM2/store use it consistently.

Sharding: pure data-parallel over batch: 16 batches -> 8 cores x 2.
"""

import sys

if "/opt/trn_rl_repo" not in sys.path:
    sys.path.insert(0, "/opt/trn_rl_repo")

import numpy as np
import ml_dtypes

import concourse.bacc as bacc
import concourse.mybir as mybir
import concourse.tile as tile
from concourse.alu_op_type import AluOpType
from concourse.bass_utils import run_bass_kernel_spmd

F32 = mybir.dt.float32
BF16 = mybir.dt.bfloat16
FP8 = mybir.dt.float8e4
AF = mybir.ActivationFunctionType

N_CORES = 8
B_FULL, H, W, C = 16, 64, 64, 512
N = H * W                      # 4096 spatial positions per batch
B_LOC = B_FULL // N_CORES      # 2 batches per core
NM = C // 128                  # 4 channel blocks
NKC = N // 256                 # 16 interleaved 256-row chunks per batch
QL = 4                         # chunks per load DMA
QS = 2                         # groups per store DMA


def build_nc(b_loc=B_LOC, n=N, c=C, num_devices=N_CORES, reps=None,
             ablate=None, staggered=True, unroll=4,
             mr_cycle="aav", ftc_cycle="av", g_cycle="av", mir_cycle="a",
             fsk_cycle="av", cast_cycle="vva", dma_cast=True,
             **_legacy):
    """Build + compile the per-core Bass program.

    reps: if set, wrap the body in a hardware For_i loop (timing builds).
    *_cycle: per-site engine rotation strings (v=DVE, a=ACT).
    """
    nkc = n // 256   # interleaved 256-row chunks
    nm = c // 128
    nql = nkc // QL  # load DMAs per batch (per dtype)

    nc = bacc.Bacc(
        "TRN2",
        target_bir_lowering=False,
        debug=False,
        num_devices=num_devices,
    )

    x_d = nc.dram_tensor("x", [b_loc * n, c], F32, kind="ExternalInput")
    gam_d = nc.dram_tensor("gamma", [1, 1], F32, kind="ExternalInput")
    id_d = nc.dram_tensor("ident", [c, c], BF16, kind="ExternalInput")
    y_d = nc.dram_tensor("y", [b_loc * n, c], BF16, kind="ExternalOutput")

    ENG = {"v": nc.vector, "a": nc.scalar}

    with tile.TileContext(nc) as tc:
        with (
            tc.tile_pool(name="fbc", bufs=2 * nql + 1) as p_fb,  # bf16 4-chunk
            tc.tile_pool(name="f8c", bufs=3) as p_f8,      # fp8 2-chunk
            tc.tile_pool(name="ft", bufs=2) as p_ft,       # f^T fp8 per batch
            tc.tile_pool(name="gsb", bufs=2 * nm) as p_g,
            tc.tile_pool(name="esb", bufs=nm) as p_e,
            tc.tile_pool(name="r8", bufs=4) as p_r8,       # fp8 R rows (paired)
            tc.tile_pool(name="stat", bufs=8 * nm) as p_stat,
            tc.tile_pool(name="outp", bufs=5) as p_out,
            tc.tile_pool(name="const", bufs=1) as p_const,
            tc.tile_pool(name="psg", bufs=1, space="PSUM") as p_psg,
            tc.tile_pool(name="pst", bufs=2, space="PSUM") as p_pst,
            tc.tile_pool(name="pso", bufs=2, space="PSUM") as p_pso,
        ):
            # --- constants (outside the timing loop) ---
            ident_rows = []
            for m in range(nm):
                t2 = p_const.tile([128, c], BF16, tag=f"id{m}",
                                  name=f"id{m}")
                nc.sync.dma_start(out=t2[:, :],
                                  in_=id_d[m * 128:(m + 1) * 128, :])
                ident_rows.append(t2)
            ident128 = ident_rows[0][:, 0:128]
            gam1 = p_const.tile([1, 1], F32, tag="gam1", name="gam1")
            nc.sync.dma_start(out=gam1[:, :], in_=gam_d[:, :])
            gamb = p_const.tile([128, 1], F32, tag="gamb", name="gamb")
            nc.gpsimd.partition_broadcast(gamb[:, :], gam1[:, :])
            # gamma * I (bf16) for the r8 build, and the 1 + gamma scalar
            igam = []
            for m in range(nm):
                t3 = p_const.tile([128, c], BF16, tag=f"ig{m}",
                                  name=f"ig{m}")
                nc.vector.tensor_scalar(
                    t3[:, :], ident_rows[m][:, :], gamb[:, :], None,
                    op0=AluOpType.mult)
                igam.append(t3)
            sgam = p_const.tile([128, 1], F32, tag="sgam", name="sgam")
            nc.vector.tensor_scalar(
                sgam[:, :], gamb[:, :], 1.0, None, op0=AluOpType.add)

            ctrs = {}

            def rot(site, cycle):
                i = ctrs.get(site, 0)
                ctrs[site] = i + 1
                return ENG[cycle[i % len(cycle)]]

            def rot_copy(site, cycle):
                eng = rot(site, cycle)
                return eng.copy if eng is nc.scalar else eng.tensor_copy

            def load4(b, ki, dt):
                """One SWDGE cast-DMA covering QL interleaved chunks."""
                base = b * n + ki * QL * 256
                src = x_d[base:base + QL * 256, :].rearrange(
                    "(q p j) c1 -> p q j c1", q=QL, p=128)
                t = (p_fb if dt == BF16 else p_f8).tile(
                    [128, QL, 2, c], dt, tag="fbc" if dt == BF16 else "f8c",
                    name=f"{'fb' if dt == BF16 else 'f8'}{b}_{ki}")
                nc.gpsimd.dma_start(out=t[:, :, :, :], in_=src)
                return t

            def gram_part(b, k, f8s, psg_rows):
                """Gram accumulation for chunk k (f8s: [128, 2, c] slice)."""
                for m in range(nm if ablate != "nogram" else 0):
                    lo = m * 128
                    nc.tensor.matmul(
                        psg_rows[m][:, 0:c - lo],
                        f8s[:, :, m * 128:(m + 1) * 128],
                        f8s[:, :, lo:c],
                        start=(k == 0),
                        stop=(k == nkc - 1),
                        perf_mode=mybir.MatmulPerfMode.DoubleRow,
                    )

            def tpose_part(b, k, fbs, ft8):
                """8 bf16 transposes per chunk; PSUM->SBUF copy casts fp8."""
                ps_t = p_pst.tile([128, 2, c], BF16, tag="pst",
                                  name=f"pst{b}_{k}")
                for j in range(2):
                    for m in range(nm):
                        nc.tensor.transpose(
                            ps_t[:, j, m * 128:(m + 1) * 128],
                            fbs[:, j, m * 128:(m + 1) * 128],
                            ident128,
                        )
                rot_copy("ftc", ftc_cycle)(ft8[:, k, :, :], ps_t[:, :, :])

            def cast8(b, ki, fb4):
                """fp8 copy of one QL-chunk slab: SWDGE sbuf->sbuf cast
                DMA (keeps the vector engines out of it) or ACT/DVE copy."""
                f8t = p_f8.tile([128, QL, 2, c], FP8, tag="f8c",
                                name=f"f8{b}_{ki}")
                if dma_cast:
                    nc.gpsimd.dma_start(out=f8t[:, :, :, :],
                                        in_=fb4[:, :, :, :])
                else:
                    rot_copy("cast", cast_cycle)(f8t[:, :, :, :],
                                                 fb4[:, :, :, :])
                return f8t

            def chunk_work(b, k, fb4, f84, ft8, psg_rows):
                q = k % QL
                f8s = f84[:, q, :, :]
                fbs = fb4[:, q, :, :]
                gram_part(b, k, f8s, psg_rows)
                tpose_part(b, k, fbs, ft8)

            def gram_finish(b, psg_rows):
                """PSUM G -> SBUF (bf16), mirror lower blocks, softmax ->
                R8 (fp8, gamma-scaled, zero diagonal, channel-pair packed)
                and the per-channel scale vector s_bc (bf16, broadcast)."""
                g_sb = []
                r8 = [p_r8.tile([128, 2, c], FP8, tag="r8",
                                name=f"r8_{b}_{t}") for t in range(2)]
                rec_st = p_stat.tile([128, nm], F32, tag="recst",
                                     name=f"recst{b}")

                def softmax_row(m):
                    t_g = g_sb[m]
                    nmax = p_stat.tile([128, 1], F32, tag="nmax",
                                       name=f"nmax{b}_{m}")
                    nc.vector.reduce_max(
                        nmax[:, :], t_g[:, :], axis=mybir.AxisListType.X,
                        negate=True,
                    )
                    e_sb = p_e.tile([128, c], BF16, tag="esb",
                                    name=f"e{b}_{m}")
                    esum = p_stat.tile([128, 1], F32, tag="esum",
                                       name=f"esum{b}_{m}")
                    nc.scalar.activation(
                        e_sb[:, :], t_g[:, :], AF.Exp,
                        bias=nmax[:, :], scale=1.0, accum_out=esum[:, :],
                    )
                    nc.vector.reciprocal(rec_st[:, m:m + 1], esum[:, :])
                    sc = p_stat.tile([128, 1], F32, tag="sc",
                                     name=f"sc{b}_{m}")
                    nc.vector.tensor_tensor(
                        sc[:, :], rec_st[:, m:m + 1], gamb[:, :],
                        op=AluOpType.mult,
                    )
                    # R8 row m: sc*E - gamma*I  (fp8).  Off-diagonal is
                    # gamma*A; the diagonal is gamma*(1/esum - 1), i.e. the
                    # per-channel correction to the scalar 1+gamma applied
                    # on the elementwise path (exactly 0 when softmax is
                    # saturated, since esum == 1).
                    nc.vector.scalar_tensor_tensor(
                        r8[m // 2][:, m % 2, :], e_sb[:, :], sc[:, :],
                        igam[m][:, :],
                        op0=AluOpType.mult, op1=AluOpType.subtract,
                    )

                for m in range(nm):
                    lo = m * 128
                    t_g = p_g.tile([128, c], BF16, tag="gsb", name=f"g{b}_{m}")
                    rot_copy("gcp", g_cycle)(t_g[:, lo:c],
                                             psg_rows[m][:, 0:c - lo])
                    g_sb.append(t_g)
                    if m:
                        tp = p_pst.tile([128, 2, c], BF16, tag="pst",
                                        name=f"gt{b}_{m}")
                        for d in range(m):
                            nc.tensor.transpose(
                                tp[:, 0, d * 128:(d + 1) * 128],
                                g_sb[d][:, m * 128:(m + 1) * 128],
                                ident128,
                            )
                        rot_copy("mir", mir_cycle)(
                            t_g[:, 0:m * 128], tp[:, 0, 0:m * 128])
                    softmax_row(m)

                return r8

            def make_fsk(b, ki, fb4):
                """fb4 <- fb4 * (1 + gamma) in place (per-partition scalar;
                ACT mul or DVE tensor_scalar, both 2-byte fast paths)."""
                for h in range(QL // 2):
                    eng = rot("fsk", fsk_cycle)
                    sl = fb4[:, 2 * h:2 * h + 2, :, :]
                    if eng is nc.scalar:
                        eng.mul(sl, sl, sgam[:, :])
                    else:
                        eng.tensor_scalar(sl, sl, sgam[:, :], None,
                                          op0=AluOpType.mult)

            def mm2_pair(b, kp, fb4s, ft8, r8):
                """MM2 + store for a pair of 256-row groups (k=2kp, 2kp+1):
                per-j fp8 DoubleRow matmuls -> f32 PSUM, ACT/DVE copy to
                m_sb (bf16), wide DVE add of the in-place fsk, one store."""
                m_sb = p_out.tile([128, QS, 2, c], BF16, tag="msb",
                                  name=f"m{b}_{kp}")
                for g in range(QS):
                    k = kp * QS + g
                    for j in range(2):
                        ps_o = p_pso.tile([128, c], F32, tag="pso",
                                          name=f"pso{b}_{k}_{j}")
                        for t in range(2):
                            nc.tensor.matmul(
                                ps_o[:, :],
                                ft8[:, k, j, t * 256:(t + 1) * 256]
                                    .rearrange("p (i q) -> p i q", i=2),
                                r8[t][:, :, :],
                                start=(t == 0),
                                stop=(t == 1),
                                perf_mode=mybir.MatmulPerfMode.DoubleRow,
                            )
                        rot_copy("mr", mr_cycle)(m_sb[:, g, j, :],
                                                 ps_o[:, :])
                # residual: m += f * (1 + gamma)  (f pre-scaled in place)
                nc.vector.tensor_tensor(
                    m_sb[:, :, :, :], m_sb[:, :, :, :], fb4s,
                    op=AluOpType.add,
                )
                base = b * n + kp * QS * 256
                nc.sync.dma_start(
                    out=y_d[base:base + QS * 256, :].rearrange(
                        "(q p j) c1 -> p q j c1", q=QS, p=128),
                    in_=m_sb[:, :, :, :],
                )

            def body(_iv=None):
                # ---- batch 0 stream ----
                ft8_0 = p_ft.tile([128, nkc, 2, c], FP8, tag="ft",
                                  name="ft0")
                psg0 = [p_psg.tile([128, c - m * 128], F32, tag=f"psg{m}",
                                   name=f"psg0_{m}") for m in range(nm)]
                fb0, f80 = [], []
                for ki in range(nql):
                    fb0.append(load4(0, ki, BF16))
                    f80.append(cast8(0, ki, fb0[ki]))
                if ablate == "loads":
                    for ki in range(nql):
                        f8k = p_f8.tile([128, QL, 2, c], FP8, tag="f8c",
                                        name=f"f8d0_{ki}")
                        nc.vector.tensor_copy(f8k[:, :, :, :],
                                              fb0[ki][:, :, :, :])
                    return
                fb1 = [load4(1, ki, BF16) for ki in range(nql)]
                f81 = [cast8(1, ki, fb1[ki]) for ki in range(nql)]
                for k in range(nkc):
                    chunk_work(0, k, fb0[k // QL], f80[k // QL], ft8_0, psg0)
                if ablate in ("nogram", "nofinish"):
                    return
                r8_0 = gram_finish(0, psg0)
                if ablate == "gram":
                    return
                for ki in range(nql):
                    make_fsk(0, ki, fb0[ki])

                # ---- batch 1 stream interleaved with batch 0 MM2 ----
                ft8_1 = p_ft.tile([128, nkc, 2, c], FP8, tag="ft",
                                  name="ft1")
                psg1 = [p_psg.tile([128, c - m * 128], F32, tag=f"psg{m}",
                                   name=f"psg1_{m}") for m in range(nm)]
                for kp in range(nkc // QS // 2):
                    for k in (4 * kp, 4 * kp + 1, 4 * kp + 2, 4 * kp + 3):
                        chunk_work(1, k, fb1[k // QL], f81[k // QL],
                                   ft8_1, psg1)
                    mm2_pair(0, kp,
                             fb0[(kp * QS) // QL]
                                [:, (kp * QS) % QL:(kp * QS) % QL + QS, :, :],
                             ft8_0, r8_0)
                if ablate == "phase4":
                    return
                r8_1 = gram_finish(1, psg1)
                for ki in range(nql):
                    make_fsk(1, ki, fb1[ki])
                for kp in range(nkc // QS // 2, nkc // QS):
                    mm2_pair(0, kp,
                             fb0[(kp * QS) // QL]
                                [:, (kp * QS) % QL:(kp * QS) % QL + QS, :, :],
                             ft8_0, r8_0)
                if ablate == "fin1":
                    return
                for kp in range(nkc // QS):
                    mm2_pair(1, kp,
                             fb1[(kp * QS) // QL]
                                [:, (kp * QS) % QL:(kp * QS) % QL + QS, :, :],
                             ft8_1, r8_1)

            if reps is None:
                body()
            else:
                assert reps % unroll == 0
                with tc.For_i(0, reps // unroll, 1,
                              staggered_reset=staggered,
                              hint_engines=(mybir.EngineType.PE,
                                            mybir.EngineType.DVE,
                                            mybir.EngineType.Activation,
                                            mybir.EngineType.Pool,
                                            mybir.EngineType.SP)) as iv:
                    for _ in range(unroll):
                        body(iv)

    nc.compile()
    return nc


_NC_CACHE = {}


def _get_nc():
    if "full" not in _NC_CACHE:
        _NC_CACHE["full"] = build_nc()
    return _NC_CACHE["full"]


def make_in_maps(inputs_np, gamma_np):
    """Shard full inputs into per-core in_maps."""
    x = np.ascontiguousarray(
        np.asarray(inputs_np, dtype=np.float32).reshape(B_FULL, N, C)
    )
    gam = np.asarray(gamma_np, dtype=np.float32).reshape(1, 1)
    ident = np.eye(C, dtype=np.float32).astype(ml_dtypes.bfloat16)
    in_maps = []
    for core in range(N_CORES):
        xs = x[core * B_LOC:(core + 1) * B_LOC].reshape(B_LOC * N, C)
        in_maps.append({
            "x": np.ascontiguousarray(xs),
            "gamma": gam,
            "ident": ident,
        })
    return in_maps


def kernel(inputs, gamma):
    nc = _get_nc()
    in_maps = make_in_maps(inputs, gamma)
    res = run_bass_kernel_spmd(nc, in_maps, core_ids=list(range(N_CORES)))
    outs = [np.asarray(res.results[c]["y"], dtype=np.float32)
            .reshape(B_LOC, N, C) for c in range(N_CORES)]
    y = np.concatenate(outs, axis=0).reshape(B_FULL, H, W, C)
    return y.astype(np.float32)


# revision 24
# speedup vs baseline: 1.1020x; 1.1020x over previous
"""Trainium2 Bass kernel for nn_CAM (channel attention module).

Reference (per batch b):
    f = x[b].reshape(N, C)                      # N = H*W = 4096, C = 512
    G = f^T f                                   # (C, C) channel gram
    A = softmax(G, axis=-1)
    out[b] = gamma * (f @ A) + x[b]

Algebraic folds:
  * out = f (.) s  +  f @ (gamma * R),  where s_d = 1 + gamma * A[d,d] and
    R = A - diag(A).  The gram's diagonal is ~N >> off-diag ~sqrt(N), so
    softmax concentrates on the diagonal; the dominant diagonal term is
    computed exactly on the elementwise path (bf16 f), and only the tiny
    off-diagonal remainder goes through the second matmul, which therefore
    runs in fp8 DoubleRow (2x bf16 PE throughput) with no accuracy cost.
    A[d,d] = 1/esum_d exactly, since exp(G_dd - rowmax) = exp(0) = 1.
  * symmetry: G == G^T, so only upper-triangular 128-blocks are computed
    (free dims 512/384/256/128); the 6 lower blocks are PE-transposed back.

Engine economy (the binding constraint once MM2 is fp8):
  * x is cast-LOADED TWICE by the SWDGE (fp32->bf16 and fp32->fp8), so no
    on-chip cast instructions exist at all; HBM has ~2.6x headroom.
  * Loads/stores are batched 4 chunks / 2 groups per DMA so the Pool/SP
    sequencers spend ~12us, not ~25us, issuing DMAs.
  * GPSIMD (Pool) executes NO tensor ops (hardware runs them far below the
    cost model's estimate); it only triggers DMAs + partition_broadcast.
  * fsk = f (.) s is computed in place over the bf16 chunk (all-SBUF
    all-bf16: DVE 2x/4x modes); the MM2 drain is an ACT/DVE plain PSUM
    copy and the residual add is a wide all-bf16 DVE tensor_tensor.

Layout: n rows are interleaved 2-per-partition (row 256k + 2p + j lives on
partition p, slice j of chunk k): 4KB load / 2KB store descriptors.  The
gram is invariant to the n-permutation; ft/MM2/store use it consistently.

Sharding: pure data-parallel over batch: 16 batches -> 8 cores x 2.
"""

import sys

if "/opt/trn_rl_repo" not in sys.path:
    sys.path.insert(0, "/opt/trn_rl_repo")

import numpy as np
import ml_dtypes

import concourse.bacc as bacc
import concourse.mybir as mybir
import concourse.tile as tile
from concourse.alu_op_type import AluOpType
from concourse.bass_utils import run_bass_kernel_spmd

F32 = mybir.dt.float32
BF16 = mybir.dt.bfloat16
FP8 = mybir.dt.float8e4
AF = mybir.ActivationFunctionType

N_CORES = 8
B_FULL, H, W, C = 16, 64, 64, 512
N = H * W                      # 4096 spatial positions per batch
B_LOC = B_FULL // N_CORES      # 2 batches per core
NM = C // 128                  # 4 channel blocks
NKC = N // 256                 # 16 interleaved 256-row chunks per batch
QL = 4                         # chunks per load DMA
QS = 2                         # groups per store DMA


def build_nc(b_loc=B_LOC, n=N, c=C, num_devices=N_CORES, reps=None,
             ablate=None, staggered=True, unroll=4,
             mr_cycle="aav", ftc_cycle="av", g_cycle="av", mir_cycle="a",
             fsk_cycle="av", cast_cycle="vva", dma_cast=False,
             **_legacy):
    """Build + compile the per-core Bass program.

    reps: if set, wrap the body in a hardware For_i loop (timing builds).
    *_cycle: per-site engine rotation strings (v=DVE, a=ACT).
    """
    nkc = n // 256   # interleaved 256-row chunks
    nm = c // 128
    nql = nkc // QL  # load DMAs per batch (per dtype)

    nc = bacc.Bacc(
        "TRN2",
        target_bir_lowering=False,
        debug=False,
        num_devices=num_devices,
    )

    x_d = nc.dram_tensor("x", [b_loc * n, c], F32, kind="ExternalInput")
    gam_d = nc.dram_tensor("gamma", [1, 1], F32, kind="ExternalInput")
    id_d = nc.dram_tensor("ident", [c, c], BF16, kind="ExternalInput")
    y_d = nc.dram_tensor("y", [b_loc * n, c], BF16, kind="ExternalOutput")

    ENG = {"v": nc.vector, "a": nc.scalar}

    with tile.TileContext(nc) as tc:
        with (
            tc.tile_pool(name="fbc", bufs=2 * nql + 1) as p_fb,  # bf16 4-chunk
            tc.tile_pool(name="f8c", bufs=3) as p_f8,      # fp8 2-chunk
            tc.tile_pool(name="ft", bufs=2) as p_ft,       # f^T fp8 per batch
            tc.tile_pool(name="gsb", bufs=2 * nm) as p_g,
            tc.tile_pool(name="esb", bufs=nm) as p_e,
            tc.tile_pool(name="r8", bufs=4) as p_r8,       # fp8 R rows (paired)
            tc.tile_pool(name="stat", bufs=8 * nm) as p_stat,
            tc.tile_pool(name="outp", bufs=5) as p_out,
            tc.tile_pool(name="const", bufs=1) as p_const,
            tc.tile_pool(name="psg", bufs=1, space="PSUM") as p_psg,
            tc.tile_pool(name="pst", bufs=2, space="PSUM") as p_pst,
            tc.tile_pool(name="pso", bufs=2, space="PSUM") as p_pso,
        ):
            # --- constants (outside the timing loop) ---
            ident_rows = []
            for m in range(nm):
                t2 = p_const.tile([128, c], BF16, tag=f"id{m}",
                                  name=f"id{m}")
                nc.sync.dma_start(out=t2[:, :],
                                  in_=id_d[m * 128:(m + 1) * 128, :])
                ident_rows.append(t2)
            ident128 = ident_rows[0][:, 0:128]
            gam1 = p_const.tile([1, 1], F32, tag="gam1", name="gam1")
            nc.sync.dma_start(out=gam1[:, :], in_=gam_d[:, :])
            gamb = p_const.tile([128, 1], F32, tag="gamb", name="gamb")
            nc.gpsimd.partition_broadcast(gamb[:, :], gam1[:, :])
            # gamma * I (bf16) for the r8 build, and the 1 + gamma scalar
            igam = []
            for m in range(nm):
                t3 = p_const.tile([128, c], BF16, tag=f"ig{m}",
                                  name=f"ig{m}")
                nc.vector.tensor_scalar(
                    t3[:, :], ident_rows[m][:, :], gamb[:, :], None,
                    op0=AluOpType.mult)
                igam.append(t3)
            sgam = p_const.tile([128, 1], F32, tag="sgam", name="sgam")
            nc.vector.tensor_scalar(
                sgam[:, :], gamb[:, :], 1.0, None, op0=AluOpType.add)

            ctrs = {}

            def rot(site, cycle):
                i = ctrs.get(site, 0)
                ctrs[site] = i + 1
                return ENG[cycle[i % len(cycle)]]

            def rot_copy(site, cycle):
                eng = rot(site, cycle)
                return eng.copy if eng is nc.scalar else eng.tensor_copy

            def load4(b, ki, dt):
                """One SWDGE cast-DMA covering QL interleaved chunks."""
                base = b * n + ki * QL * 256
                src = x_d[base:base + QL * 256, :].rearrange(
                    "(q p j) c1 -> p q j c1", q=QL, p=128)
                t = (p_fb if dt == BF16 else p_f8).tile(
                    [128, QL, 2, c], dt, tag="fbc" if dt == BF16 else "f8c",
                    name=f"{'fb' if dt == BF16 else 'f8'}{b}_{ki}")
                nc.gpsimd.dma_start(out=t[:, :, :, :], in_=src)
                return t

            def gram_part(b, k, f8s, psg_rows):
                """Gram accumulation for chunk k (f8s: [128, 2, c] slice)."""
                for m in range(nm if ablate != "nogram" else 0):
                    lo = m * 128
                    nc.tensor.matmul(
                        psg_rows[m][:, 0:c - lo],
                        f8s[:, :, m * 128:(m + 1) * 128],
                        f8s[:, :, lo:c],
                        start=(k == 0),
                        stop=(k == nkc - 1),
                        perf_mode=mybir.MatmulPerfMode.DoubleRow,
                    )

            def tpose_part(b, k, fbs, ft8):
                """8 bf16 transposes per chunk; PSUM->SBUF copy casts fp8."""
                ps_t = p_pst.tile([128, 2, c], BF16, tag="pst",
                                  name=f"pst{b}_{k}")
                for j in range(2):
                    for m in range(nm):
                        nc.tensor.transpose(
                            ps_t[:, j, m * 128:(m + 1) * 128],
                            fbs[:, j, m * 128:(m + 1) * 128],
                            ident128,
                        )
                rot_copy("ftc", ftc_cycle)(ft8[:, k, :, :], ps_t[:, :, :])

            def cast8(b, ki, fb4):
                """fp8 copy of one QL-chunk slab: SWDGE sbuf->sbuf cast
                DMA (keeps the vector engines out of it) or ACT/DVE copy."""
                f8t = p_f8.tile([128, QL, 2, c], FP8, tag="f8c",
                                name=f"f8{b}_{ki}")
                if dma_cast:
                    nc.gpsimd.dma_start(out=f8t[:, :, :, :],
                                        in_=fb4[:, :, :, :])
                else:
                    for h in range(QL // 2):
                        rot_copy("cast", cast_cycle)(
                            f8t[:, 2 * h:2 * h + 2, :, :],
                            fb4[:, 2 * h:2 * h + 2, :, :])
                return f8t

            def chunk_work(b, k, fb4, f84, ft8, psg_rows):
                q = k % QL
                f8s = f84[:, q, :, :]
                fbs = fb4[:, q, :, :]
                gram_part(b, k, f8s, psg_rows)
                tpose_part(b, k, fbs, ft8)

            def gram_finish(b, psg_rows):
                """PSUM G -> SBUF (bf16), mirror lower blocks, softmax ->
                R8 (fp8, gamma-scaled, zero diagonal, channel-pair packed)
                and the per-channel scale vector s_bc (bf16, broadcast)."""
                g_sb = []
                r8 = [p_r8.tile([128, 2, c], FP8, tag="r8",
                                name=f"r8_{b}_{t}") for t in range(2)]
                rec_st = p_stat.tile([128, nm], F32, tag="recst",
                                     name=f"recst{b}")

                def softmax_row(m):
                    t_g = g_sb[m]
                    nmax = p_stat.tile([128, 1], F32, tag="nmax",
                                       name=f"nmax{b}_{m}")
                    nc.vector.reduce_max(
                        nmax[:, :], t_g[:, :], axis=mybir.AxisListType.X,
                        negate=True,
                    )
                    e_sb = p_e.tile([128, c], BF16, tag="esb",
                                    name=f"e{b}_{m}")
                    esum = p_stat.tile([128, 1], F32, tag="esum",
                                       name=f"esum{b}_{m}")
                    nc.scalar.activation(
                        e_sb[:, :], t_g[:, :], AF.Exp,
                        bias=nmax[:, :], scale=1.0, accum_out=esum[:, :],
                    )
                    nc.vector.reciprocal(rec_st[:, m:m + 1], esum[:, :])
                    sc = p_stat.tile([128, 1], F32, tag="sc",
                                     name=f"sc{b}_{m}")
                    nc.vector.tensor_tensor(
                        sc[:, :], rec_st[:, m:m + 1], gamb[:, :],
                        op=AluOpType.mult,
                    )
                    # R8 row m: sc*E - gamma*I  (fp8).  Off-diagonal is
                    # gamma*A; the diagonal is gamma*(1/esum - 1), i.e. the
                    # per-channel correction to the scalar 1+gamma applied
                    # on the elementwise path (exactly 0 when softmax is
                    # saturated, since esum == 1).
                    nc.vector.scalar_tensor_tensor(
                        r8[m // 2][:, m % 2, :], e_sb[:, :], sc[:, :],
                        igam[m][:, :],
                        op0=AluOpType.mult, op1=AluOpType.subtract,
                    )

                for m in range(nm):
                    lo = m * 128
                    t_g = p_g.tile([128, c], BF16, tag="gsb", name=f"g{b}_{m}")
                    rot_copy("gcp", g_cycle)(t_g[:, lo:c],
                                             psg_rows[m][:, 0:c - lo])
                    g_sb.append(t_g)
                    if m:
                        tp = p_pst.tile([128, 2, c], BF16, tag="pst",
                                        name=f"gt{b}_{m}")
                        for d in range(m):
                            nc.tensor.transpose(
                                tp[:, 0, d * 128:(d + 1) * 128],
                                g_sb[d][:, m * 128:(m + 1) * 128],
                                ident128,
                            )
                        rot_copy("mir", mir_cycle)(
                            t_g[:, 0:m * 128], tp[:, 0, 0:m * 128])
                    softmax_row(m)

                return r8

            def make_fsk(b, ki, fb4):
                """fb4 <- fb4 * (1 + gamma) in place (per-partition scalar;
                ACT mul or DVE tensor_scalar, both 2-byte fast paths)."""
                for h in range(QL // 2):
                    eng = rot("fsk", fsk_cycle)
                    sl = fb4[:, 2 * h:2 * h + 2, :, :]
                    if eng is nc.scalar:
                        eng.mul(sl, sl, sgam[:, :])
                    else:
                        eng.tensor_scalar(sl, sl, sgam[:, :], None,
                                          op0=AluOpType.mult)

            def mm2_pair(b, kp, fb4s, ft8, r8):
                """MM2 + store for a pair of 256-row groups (k=2kp, 2kp+1):
                per-j fp8 DoubleRow matmuls -> f32 PSUM, ACT/DVE copy to
                m_sb (bf16), wide DVE add of the in-place fsk, one store."""
                m_sb = p_out.tile([128, QS, 2, c], BF16, tag="msb",
                                  name=f"m{b}_{kp}")
                for g in range(QS):
                    k = kp * QS + g
                    for j in range(2):
                        ps_o = p_pso.tile([128, c], F32, tag="pso",
                                          name=f"pso{b}_{k}_{j}")
                        for t in range(2):
                            nc.tensor.matmul(
                                ps_o[:, :],
                                ft8[:, k, j, t * 256:(t + 1) * 256]
                                    .rearrange("p (i q) -> p i q", i=2),
                                r8[t][:, :, :],
                                start=(t == 0),
                                stop=(t == 1),
                                perf_mode=mybir.MatmulPerfMode.DoubleRow,
                            )
                        rot_copy("mr", mr_cycle)(m_sb[:, g, j, :],
                                                 ps_o[:, :])
                # residual: m += f * (1 + gamma)  (f pre-scaled in place)
                nc.vector.tensor_tensor(
                    m_sb[:, :, :, :], m_sb[:, :, :, :], fb4s,
                    op=AluOpType.add,
                )
                base = b * n + kp * QS * 256
                nc.sync.dma_start(
                    out=y_d[base:base + QS * 256, :].rearrange(
                        "(q p j) c1 -> p q j c1", q=QS, p=128),
                    in_=m_sb[:, :, :, :],
                )

            def body(_iv=None):
                # ---- batch 0 stream ----
                ft8_0 = p_ft.tile([128, nkc, 2, c], FP8, tag="ft",
                                  name="ft0")
                psg0 = [p_psg.tile([128, c - m * 128], F32, tag=f"psg{m}",
                                   name=f"psg0_{m}") for m in range(nm)]
                fb0, f80 = [], []
                for ki in range(nql):
                    fb0.append(load4(0, ki, BF16))
                    f80.append(cast8(0, ki, fb0[ki]))
                if ablate == "loads":
                    for ki in range(nql):
                        f8k = p_f8.tile([128, QL, 2, c], FP8, tag="f8c",
                                        name=f"f8d0_{ki}")
                        nc.vector.tensor_copy(f8k[:, :, :, :],
                                              fb0[ki][:, :, :, :])
                    return
                fb1 = [load4(1, ki, BF16) for ki in range(nql)]
                f81 = [cast8(1, ki, fb1[ki]) for ki in range(nql)]
                for k in range(nkc):
                    chunk_work(0, k, fb0[k // QL], f80[k // QL], ft8_0, psg0)
                if ablate in ("nogram", "nofinish"):
                    return
                r8_0 = gram_finish(0, psg0)
                if ablate == "gram":
                    return
                for ki in range(nql):
                    make_fsk(0, ki, fb0[ki])

                # ---- batch 1 stream interleaved with batch 0 MM2 ----
                ft8_1 = p_ft.tile([128, nkc, 2, c], FP8, tag="ft",
                                  name="ft1")
                psg1 = [p_psg.tile([128, c - m * 128], F32, tag=f"psg{m}",
                                   name=f"psg1_{m}") for m in range(nm)]
                for kp in range(nkc // QS // 2):
                    for k in (4 * kp, 4 * kp + 1, 4 * kp + 2, 4 * kp + 3):
                        chunk_work(1, k, fb1[k // QL], f81[k // QL],
                                   ft8_1, psg1)
                    mm2_pair(0, kp,
                             fb0[(kp * QS) // QL]
                                [:, (kp * QS) % QL:(kp * QS) % QL + QS, :, :],
                             ft8_0, r8_0)
                if ablate == "phase4":
                    return
                r8_1 = gram_finish(1, psg1)
                for ki in range(nql):
                    make_fsk(1, ki, fb1[ki])
                for kp in range(nkc // QS // 2, nkc // QS):
                    mm2_pair(0, kp,
                             fb0[(kp * QS) // QL]
                                [:, (kp * QS) % QL:(kp * QS) % QL + QS, :, :],
                             ft8_0, r8_0)
                if ablate == "fin1":
                    return
                for kp in range(nkc // QS):
                    mm2_pair(1, kp,
                             fb1[(kp * QS) // QL]
                                [:, (kp * QS) % QL:(kp * QS) % QL + QS, :, :],
                             ft8_1, r8_1)

            if reps is None:
                body()
            else:
                assert reps % unroll == 0
                with tc.For_i(0, reps // unroll, 1,
                              staggered_reset=staggered,
                              hint_engines=(mybir.EngineType.PE,
                                            mybir.EngineType.DVE,
                                            mybir.EngineType.Activation,
                                            mybir.EngineType.Pool,
                                            mybir.EngineType.SP)) as iv:
                    for _ in range(unroll):
                        body(iv)

    nc.compile()
    return nc


_NC_CACHE = {}


def _get_nc():
    if "full" not in _NC_CACHE:
        _NC_CACHE["full"] = build_nc()
    return _NC_CACHE["full"]


def make_in_maps(inputs_np, gamma_np):
    """Shard full inputs into per-core in_maps."""
    x = np.ascontiguousarray(
        np.asarray(inputs_np, dtype=np.float32).reshape(B_FULL, N, C)
    )
    gam = np.asarray(gamma_np, dtype=np.float32).reshape(1, 1)
    ident = np.eye(C, dtype=np.float32).astype(ml_dtypes.bfloat16)
    in_maps = []
    for core in range(N_CORES):
        xs = x[core * B_LOC:(core + 1) * B_LOC].reshape(B_LOC * N, C)
        in_maps.append({
            "x": np.ascontiguousarray(xs),
            "gamma": gam,
            "ident": ident,
        })
    return in_maps


def kernel(inputs, gamma):
    nc = _get_nc()
    in_maps = make_in_maps(inputs, gamma)
    res = run_bass_kernel_spmd(nc, in_maps, core_ids=list(range(N_CORES)))
    outs = [np.asarray(res.results[c]["y"], dtype=np.float32)
            .reshape(B_LOC, N, C) for c in range(N_CORES)]
    y = np.concatenate(outs, axis=0).reshape(B_FULL, H, W, C)
    return y.astype(np.float32)


# revision 25
# speedup vs baseline: 1.1859x; 1.0761x over previous
"""Trainium2 Bass kernel for nn_CAM (channel attention module).

Reference (per batch b):
    f = x[b].reshape(N, C)                      # N = H*W = 4096, C = 512
    G = f^T f                                   # (C, C) channel gram
    A = softmax(G, axis=-1)
    out[b] = gamma * (f @ A) + x[b]

Algebraic folds:
  * out = f (.) s  +  f @ (gamma * R),  where s_d = 1 + gamma * A[d,d] and
    R = A - diag(A).  The gram's diagonal is ~N >> off-diag ~sqrt(N), so
    softmax concentrates on the diagonal; the dominant diagonal term is
    computed exactly on the elementwise path (bf16 f), and only the tiny
    off-diagonal remainder goes through the second matmul, which therefore
    runs in fp8 DoubleRow (2x bf16 PE throughput) with no accuracy cost.
    A[d,d] = 1/esum_d exactly, since exp(G_dd - rowmax) = exp(0) = 1.
  * symmetry: G == G^T, so only upper-triangular 128-blocks are computed
    (free dims 512/384/256/128); the 6 lower blocks are PE-transposed back.

Engine economy (the binding constraint once MM2 is fp8):
  * x is cast-LOADED TWICE by the SWDGE (fp32->bf16 and fp32->fp8), so no
    on-chip cast instructions exist at all; HBM has ~2.6x headroom.
  * Loads/stores are batched 4 chunks / 2 groups per DMA so the Pool/SP
    sequencers spend ~12us, not ~25us, issuing DMAs.
  * GPSIMD (Pool) executes NO tensor ops (hardware runs them far below the
    cost model's estimate); it only triggers DMAs + partition_broadcast.
  * fsk = f (.) s is computed in place over the bf16 chunk (all-SBUF
    all-bf16: DVE 2x/4x modes); the MM2 drain is an ACT/DVE plain PSUM
    copy and the residual add is a wide all-bf16 DVE tensor_tensor.

Layout: n rows are interleaved 2-per-partition (row 256k + 2p + j lives on
partition p, slice j of chunk k): 4KB load / 2KB store descriptors.  The
gram is invariant to the n-permutation; ft/MM2/store use it consistently.

Sharding: pure data-parallel over batch: 16 batches -> 8 cores x 2.
"""

import sys

if "/opt/trn_rl_repo" not in sys.path:
    sys.path.insert(0, "/opt/trn_rl_repo")

import numpy as np
import ml_dtypes

import concourse.bacc as bacc
import concourse.mybir as mybir
import concourse.tile as tile
from concourse.alu_op_type import AluOpType
from concourse.bass_utils import run_bass_kernel_spmd

F32 = mybir.dt.float32
BF16 = mybir.dt.bfloat16
FP8 = mybir.dt.float8e4
AF = mybir.ActivationFunctionType

N_CORES = 8
B_FULL, H, W, C = 16, 64, 64, 512
N = H * W                      # 4096 spatial positions per batch
B_LOC = B_FULL // N_CORES      # 2 batches per core
NM = C // 128                  # 4 channel blocks
NKC = N // 256                 # 16 interleaved 256-row chunks per batch
QL = 4                         # chunks per load DMA
QS = 2                         # groups per store DMA


def build_nc(b_loc=B_LOC, n=N, c=C, num_devices=N_CORES, reps=None,
             ablate=None, staggered=True, unroll=4,
             mr_cycle="aav", ftc_cycle="av", g_cycle="av", mir_cycle="a",
             fsk_cycle="av", cast_cycle="vva", dma_cast=False,
             **_legacy):
    """Build + compile the per-core Bass program.

    reps: if set, wrap the body in a hardware For_i loop (timing builds).
    *_cycle: per-site engine rotation strings (v=DVE, a=ACT).
    """
    nkc = n // 256   # interleaved 256-row chunks
    nm = c // 128
    nql = nkc // QL  # load DMAs per batch (per dtype)

    nc = bacc.Bacc(
        "TRN2",
        target_bir_lowering=False,
        debug=False,
        num_devices=num_devices,
    )

    x_d = nc.dram_tensor("x", [b_loc * n, c], F32, kind="ExternalInput")
    gam_d = nc.dram_tensor("gamma", [1, 1], F32, kind="ExternalInput")
    id_d = nc.dram_tensor("ident", [c, c], BF16, kind="ExternalInput")
    y_d = nc.dram_tensor("y", [b_loc * n, c], BF16, kind="ExternalOutput")

    ENG = {"v": nc.vector, "a": nc.scalar}

    with tile.TileContext(nc) as tc:
        with (
            tc.tile_pool(name="fbc", bufs=2 * nql + 1) as p_fb,  # bf16 4-chunk
            tc.tile_pool(name="f8c", bufs=3) as p_f8,      # fp8 2-chunk
            tc.tile_pool(name="ft", bufs=2) as p_ft,       # f^T fp8 per batch
            tc.tile_pool(name="gsb", bufs=2 * nm) as p_g,
            tc.tile_pool(name="esb", bufs=nm) as p_e,
            tc.tile_pool(name="r8", bufs=4) as p_r8,       # fp8 R rows (paired)
            tc.tile_pool(name="stat", bufs=8 * nm) as p_stat,
            tc.tile_pool(name="outp", bufs=5) as p_out,
            tc.tile_pool(name="const", bufs=1) as p_const,
            tc.tile_pool(name="psg", bufs=1, space="PSUM") as p_psg,
            tc.tile_pool(name="pst", bufs=2, space="PSUM") as p_pst,
            tc.tile_pool(name="pso", bufs=2, space="PSUM") as p_pso,
        ):
            # --- constants (outside the timing loop) ---
            ident_rows = []
            for m in range(nm):
                t2 = p_const.tile([128, c], BF16, tag=f"id{m}",
                                  name=f"id{m}")
                nc.sync.dma_start(out=t2[:, :],
                                  in_=id_d[m * 128:(m + 1) * 128, :])
                ident_rows.append(t2)
            ident128 = ident_rows[0][:, 0:128]
            gam1 = p_const.tile([1, 1], F32, tag="gam1", name="gam1")
            nc.sync.dma_start(out=gam1[:, :], in_=gam_d[:, :])
            gamb = p_const.tile([128, 1], F32, tag="gamb", name="gamb")
            nc.gpsimd.partition_broadcast(gamb[:, :], gam1[:, :])
            # gamma * I (bf16) for the r8 build, and the 1 + gamma scalar
            igam = []
            for m in range(nm):
                t3 = p_const.tile([128, c], BF16, tag=f"ig{m}",
                                  name=f"ig{m}")
                nc.vector.tensor_scalar(
                    t3[:, :], ident_rows[m][:, :], gamb[:, :], None,
                    op0=AluOpType.mult)
                igam.append(t3)
            sgam = p_const.tile([128, 1], F32, tag="sgam", name="sgam")
            nc.vector.tensor_scalar(
                sgam[:, :], gamb[:, :], 1.0, None, op0=AluOpType.add)

            ctrs = {}

            def rot(site, cycle):
                i = ctrs.get(site, 0)
                ctrs[site] = i + 1
                return ENG[cycle[i % len(cycle)]]

            def rot_copy(site, cycle):
                eng = rot(site, cycle)
                return eng.copy if eng is nc.scalar else eng.tensor_copy

            def load4(b, ki, dt):
                """One SWDGE cast-DMA covering QL interleaved chunks."""
                base = b * n + ki * QL * 256
                src = x_d[base:base + QL * 256, :].rearrange(
                    "(q p j) c1 -> p q j c1", q=QL, p=128)
                t = (p_fb if dt == BF16 else p_f8).tile(
                    [128, QL, 2, c], dt, tag="fbc" if dt == BF16 else "f8c",
                    name=f"{'fb' if dt == BF16 else 'f8'}{b}_{ki}")
                nc.gpsimd.dma_start(out=t[:, :, :, :], in_=src)
                return t

            def gram_part(b, k, f8s, psg_rows):
                """Gram accumulation for chunk k (f8s: [128, 2, c] slice)."""
                for m in range(nm if ablate != "nogram" else 0):
                    lo = m * 128
                    nc.tensor.matmul(
                        psg_rows[m][:, 0:c - lo],
                        f8s[:, :, m * 128:(m + 1) * 128],
                        f8s[:, :, lo:c],
                        start=(k == 0),
                        stop=(k == nkc - 1),
                        perf_mode=mybir.MatmulPerfMode.DoubleRow,
                    )

            def tpose_part(b, k, fbs, ft8):
                """8 bf16 transposes per chunk; PSUM->SBUF copy casts fp8."""
                ps_t = p_pst.tile([128, 2, c], BF16, tag="pst",
                                  name=f"pst{b}_{k}")
                for j in range(2):
                    for m in range(nm):
                        nc.tensor.transpose(
                            ps_t[:, j, m * 128:(m + 1) * 128],
                            fbs[:, j, m * 128:(m + 1) * 128],
                            ident128,
                        )
                rot_copy("ftc", ftc_cycle)(ft8[:, k, :, :], ps_t[:, :, :])

            def cast8(b, ki, fb4):
                """fp8 copy of one QL-chunk slab: SWDGE sbuf->sbuf cast
                DMA (keeps the vector engines out of it) or ACT/DVE copy."""
                f8t = p_f8.tile([128, QL, 2, c], FP8, tag="f8c",
                                name=f"f8{b}_{ki}")
                if dma_cast:
                    nc.gpsimd.dma_start(out=f8t[:, :, :, :],
                                        in_=fb4[:, :, :, :])
                else:
                    for h in range(QL // 2):
                        rot_copy("cast", cast_cycle)(
                            f8t[:, 2 * h:2 * h + 2, :, :],
                            fb4[:, 2 * h:2 * h + 2, :, :])
                return f8t

            def chunk_work(b, k, fb4, f84, ft8, psg_rows):
                q = k % QL
                f8s = f84[:, q, :, :]
                fbs = fb4[:, q, :, :]
                gram_part(b, k, f8s, psg_rows)
                tpose_part(b, k, fbs, ft8)

            def gram_finish(b, psg_rows):
                """PSUM G -> SBUF (bf16), mirror lower blocks, softmax ->
                R8 (fp8, gamma-scaled, zero diagonal, channel-pair packed)
                and the per-channel scale vector s_bc (bf16, broadcast)."""
                g_sb = []
                r8 = [p_r8.tile([128, 2, c], FP8, tag="r8",
                                name=f"r8_{b}_{t}") for t in range(2)]
                rec_st = p_stat.tile([128, nm], F32, tag="recst",
                                     name=f"recst{b}")

                def softmax_row(m):
                    t_g = g_sb[m]
                    nmax = p_stat.tile([128, 1], F32, tag="nmax",
                                       name=f"nmax{b}_{m}")
                    nc.vector.reduce_max(
                        nmax[:, :], t_g[:, :], axis=mybir.AxisListType.X,
                        negate=True,
                    )
                    e_sb = p_e.tile([128, c], BF16, tag="esb",
                                    name=f"e{b}_{m}")
                    esum = p_stat.tile([128, 1], F32, tag="esum",
                                       name=f"esum{b}_{m}")
                    nc.scalar.activation(
                        e_sb[:, :], t_g[:, :], AF.Exp,
                        bias=nmax[:, :], scale=1.0, accum_out=esum[:, :],
                    )
                    nc.vector.reciprocal(rec_st[:, m:m + 1], esum[:, :])
                    sc = p_stat.tile([128, 1], F32, tag="sc",
                                     name=f"sc{b}_{m}")
                    nc.vector.tensor_tensor(
                        sc[:, :], rec_st[:, m:m + 1], gamb[:, :],
                        op=AluOpType.mult,
                    )
                    # R8 row m: sc*E - gamma*I  (fp8).  Off-diagonal is
                    # gamma*A; the diagonal is gamma*(1/esum - 1), i.e. the
                    # per-channel correction to the scalar 1+gamma applied
                    # on the elementwise path (exactly 0 when softmax is
                    # saturated, since esum == 1).
                    nc.vector.scalar_tensor_tensor(
                        r8[m // 2][:, m % 2, :], e_sb[:, :], sc[:, :],
                        igam[m][:, :],
                        op0=AluOpType.mult, op1=AluOpType.subtract,
                    )

                for m in range(nm):
                    lo = m * 128
                    t_g = p_g.tile([128, c], BF16, tag="gsb", name=f"g{b}_{m}")
                    rot_copy("gcp", g_cycle)(t_g[:, lo:c],
                                             psg_rows[m][:, 0:c - lo])
                    g_sb.append(t_g)
                    if m:
                        tp = p_pst.tile([128, 2, c], BF16, tag="pst",
                                        name=f"gt{b}_{m}")
                        for d in range(m):
                            nc.tensor.transpose(
                                tp[:, 0, d * 128:(d + 1) * 128],
                                g_sb[d][:, m * 128:(m + 1) * 128],
                                ident128,
                            )
                        rot_copy("mir", mir_cycle)(
                            t_g[:, 0:m * 128], tp[:, 0, 0:m * 128])
                    softmax_row(m)

                return r8

            def make_fsk(b, ki, fb4):
                """fb4 <- fb4 * (1 + gamma) in place (per-partition scalar;
                ACT mul or DVE tensor_scalar, both 2-byte fast paths)."""
                for h in range(QL // 2):
                    eng = rot("fsk", fsk_cycle)
                    sl = fb4[:, 2 * h:2 * h + 2, :, :]
                    if eng is nc.scalar:
                        eng.mul(sl, sl, sgam[:, :])
                    else:
                        eng.tensor_scalar(sl, sl, sgam[:, :], None,
                                          op0=AluOpType.mult)

            def mm2_pair(b, kp, fb4s, ft8, r8):
                """MM2 + store for a pair of 256-row groups (k=2kp, 2kp+1):
                per-j fp8 DoubleRow matmuls -> f32 PSUM, ACT/DVE copy to
                m_sb (bf16), wide DVE add of the in-place fsk, one store."""
                m_sb = p_out.tile([128, QS, 2, c], BF16, tag="msb",
                                  name=f"m{b}_{kp}")
                for g in range(QS):
                    k = kp * QS + g
                    for j in range(2):
                        ps_o = p_pso.tile([128, c], F32, tag="pso",
                                          name=f"pso{b}_{k}_{j}")
                        for t in range(2):
                            nc.tensor.matmul(
                                ps_o[:, :],
                                ft8[:, k, j, t * 256:(t + 1) * 256]
                                    .rearrange("p (i q) -> p i q", i=2),
                                r8[t][:, :, :],
                                start=(t == 0),
                                stop=(t == 1),
                                perf_mode=mybir.MatmulPerfMode.DoubleRow,
                            )
                        rot_copy("mr", mr_cycle)(m_sb[:, g, j, :],
                                                 ps_o[:, :])
                # residual: m += f * (1 + gamma)  (f pre-scaled in place)
                nc.vector.tensor_tensor(
                    m_sb[:, :, :, :], m_sb[:, :, :, :], fb4s,
                    op=AluOpType.add,
                )
                base = b * n + kp * QS * 256
                nc.sync.dma_start(
                    out=y_d[base:base + QS * 256, :].rearrange(
                        "(q p j) c1 -> p q j c1", q=QS, p=128),
                    in_=m_sb[:, :, :, :],
                )

            def body(_iv=None):
                # ---- batch 0 stream ----
                ft8_0 = p_ft.tile([128, nkc, 2, c], FP8, tag="ft",
                                  name="ft0")
                psg0 = [p_psg.tile([128, c - m * 128], F32, tag=f"psg{m}",
                                   name=f"psg0_{m}") for m in range(nm)]
                fb0, f80 = [], []
                for ki in range(nql):
                    fb0.append(load4(0, ki, BF16))
                    f80.append(cast8(0, ki, fb0[ki]))
                if ablate == "loads":
                    for ki in range(nql):
                        f8k = p_f8.tile([128, QL, 2, c], FP8, tag="f8c",
                                        name=f"f8d0_{ki}")
                        nc.vector.tensor_copy(f8k[:, :, :, :],
                                              fb0[ki][:, :, :, :])
                    return
                fb1 = [load4(1, ki, BF16) for ki in range(nql)]
                f81 = []
                for k in range(nkc):
                    chunk_work(0, k, fb0[k // QL], f80[k // QL], ft8_0, psg0)
                if ablate in ("nogram", "nofinish"):
                    return
                r8_0 = gram_finish(0, psg0)
                if ablate == "gram":
                    return
                for ki in range(nql):
                    make_fsk(0, ki, fb0[ki])

                # ---- batch 1 stream interleaved with batch 0 MM2 ----
                ft8_1 = p_ft.tile([128, nkc, 2, c], FP8, tag="ft",
                                  name="ft1")
                psg1 = [p_psg.tile([128, c - m * 128], F32, tag=f"psg{m}",
                                   name=f"psg1_{m}") for m in range(nm)]
                for kp in range(nkc // QS // 2):
                    for k in (4 * kp, 4 * kp + 1, 4 * kp + 2, 4 * kp + 3):
                        if k % QL == 0:
                            f81.append(cast8(1, k // QL, fb1[k // QL]))
                        chunk_work(1, k, fb1[k // QL], f81[k // QL],
                                   ft8_1, psg1)
                    mm2_pair(0, kp,
                             fb0[(kp * QS) // QL]
                                [:, (kp * QS) % QL:(kp * QS) % QL + QS, :, :],
                             ft8_0, r8_0)
                if ablate == "phase4":
                    return
                r8_1 = gram_finish(1, psg1)
                for ki in range(nql):
                    make_fsk(1, ki, fb1[ki])
                for kp in range(nkc // QS // 2, nkc // QS):
                    mm2_pair(0, kp,
                             fb0[(kp * QS) // QL]
                                [:, (kp * QS) % QL:(kp * QS) % QL + QS, :, :],
                             ft8_0, r8_0)
                if ablate == "fin1":
                    return
                for kp in range(nkc // QS):
                    mm2_pair(1, kp,
                             fb1[(kp * QS) // QL]
                                [:, (kp * QS) % QL:(kp * QS) % QL + QS, :, :],
                             ft8_1, r8_1)

            if reps is None:
                body()
            else:
                assert reps % unroll == 0
                with tc.For_i(0, reps // unroll, 1,
                              staggered_reset=staggered,
                              hint_engines=(mybir.EngineType.PE,
                                            mybir.EngineType.DVE,
                                            mybir.EngineType.Activation,
                                            mybir.EngineType.Pool,
                                            mybir.EngineType.SP)) as iv:
                    for _ in range(unroll):
                        body(iv)

    nc.compile()
    return nc


_NC_CACHE = {}


def _get_nc():
    if "full" not in _NC_CACHE:
        _NC_CACHE["full"] = build_nc()
    return _NC_CACHE["full"]


def make_in_maps(inputs_np, gamma_np):
    """Shard full inputs into per-core in_maps."""
    x = np.ascontiguousarray(
        np.asarray(inputs_np, dtype=np.float32).reshape(B_FULL, N, C)
    )
    gam = np.asarray(gamma_np, dtype=np.float32).reshape(1, 1)
    ident = np.eye(C, dtype=np.float32).astype(ml_dtypes.bfloat16)
    in_maps = []
    for core in range(N_CORES):
        xs = x[core * B_LOC:(core + 1) * B_LOC].reshape(B_LOC * N, C)
        in_maps.append({
            "x": np.ascontiguousarray(xs),
            "gamma": gam,
            "ident": ident,
        })
    return in_maps


def kernel(inputs, gamma):
    nc = _get_nc()
    in_maps = make_in_maps(inputs, gamma)
    res = run_bass_kernel_spmd(nc, in_maps, core_ids=list(range(N_CORES)))
    outs = [np.asarray(res.results[c]["y"], dtype=np.float32)
            .reshape(B_LOC, N, C) for c in range(N_CORES)]
    y = np.concatenate(outs, axis=0).reshape(B_FULL, H, W, C)
    return y.astype(np.float32)


# revision 26
# speedup vs baseline: 1.2775x; 1.0772x over previous
"""Trainium2 Bass kernel for nn_CAM (channel attention module).

Reference (per batch b):
    f = x[b].reshape(N, C)                      # N = H*W = 4096, C = 512
    G = f^T f                                   # (C, C) channel gram
    A = softmax(G, axis=-1)
    out[b] = gamma * (f @ A) + x[b]

Algebraic folds:
  * out = f (.) s  +  f @ (gamma * R),  where s_d = 1 + gamma * A[d,d] and
    R = A - diag(A).  The gram's diagonal is ~N >> off-diag ~sqrt(N), so
    softmax concentrates on the diagonal; the dominant diagonal term is
    computed exactly on the elementwise path (bf16 f), and only the tiny
    off-diagonal remainder goes through the second matmul, which therefore
    runs in fp8 DoubleRow (2x bf16 PE throughput) with no accuracy cost.
    A[d,d] = 1/esum_d exactly, since exp(G_dd - rowmax) = exp(0) = 1.
  * symmetry: G == G^T, so only upper-triangular 128-blocks are computed
    (free dims 512/384/256/128); the 6 lower blocks are PE-transposed back.

Engine economy (the binding constraint once MM2 is fp8):
  * x is cast-LOADED TWICE by the SWDGE (fp32->bf16 and fp32->fp8), so no
    on-chip cast instructions exist at all; HBM has ~2.6x headroom.
  * Loads/stores are batched 4 chunks / 2 groups per DMA so the Pool/SP
    sequencers spend ~12us, not ~25us, issuing DMAs.
  * GPSIMD (Pool) executes NO tensor ops (hardware runs them far below the
    cost model's estimate); it only triggers DMAs + partition_broadcast.
  * fsk = f (.) s is computed in place over the bf16 chunk (all-SBUF
    all-bf16: DVE 2x/4x modes); the MM2 drain is an ACT/DVE plain PSUM
    copy and the residual add is a wide all-bf16 DVE tensor_tensor.

Layout: n rows are interleaved 2-per-partition (row 256k + 2p + j lives on
partition p, slice j of chunk k): 4KB load / 2KB store descriptors.  The
gram is invariant to the n-permutation; ft/MM2/store use it consistently.

Sharding: pure data-parallel over batch: 16 batches -> 8 cores x 2.
"""

import sys

if "/opt/trn_rl_repo" not in sys.path:
    sys.path.insert(0, "/opt/trn_rl_repo")

import numpy as np
import ml_dtypes

import concourse.bacc as bacc
import concourse.mybir as mybir
import concourse.tile as tile
from concourse.alu_op_type import AluOpType
from concourse.bass_utils import run_bass_kernel_spmd

F32 = mybir.dt.float32
BF16 = mybir.dt.bfloat16
FP8 = mybir.dt.float8e4
AF = mybir.ActivationFunctionType

N_CORES = 8
B_FULL, H, W, C = 16, 64, 64, 512
N = H * W                      # 4096 spatial positions per batch
B_LOC = B_FULL // N_CORES      # 2 batches per core
NM = C // 128                  # 4 channel blocks
NKC = N // 256                 # 16 interleaved 256-row chunks per batch
QL = 4                         # chunks per load DMA
QS = 2                         # groups per store DMA


def build_nc(b_loc=B_LOC, n=N, c=C, num_devices=N_CORES, reps=None,
             ablate=None, staggered=True, unroll=4,
             mr_cycle="aav", ftc_cycle="av", g_cycle="av", mir_cycle="a",
             fsk_cycle="av", cast_cycle="vva", dma_cast=False,
             **_legacy):
    """Build + compile the per-core Bass program.

    reps: if set, wrap the body in a hardware For_i loop (timing builds).
    *_cycle: per-site engine rotation strings (v=DVE, a=ACT).
    """
    nkc = n // 256   # interleaved 256-row chunks
    nm = c // 128
    nql = nkc // QL  # load DMAs per batch (per dtype)

    nc = bacc.Bacc(
        "TRN2",
        target_bir_lowering=False,
        debug=False,
        num_devices=num_devices,
    )

    x_d = nc.dram_tensor("x", [b_loc * n, c], F32, kind="ExternalInput")
    gam_d = nc.dram_tensor("gamma", [1, 1], F32, kind="ExternalInput")
    id_d = nc.dram_tensor("ident", [c, c], BF16, kind="ExternalInput")
    y_d = nc.dram_tensor("y", [b_loc * n, c], BF16, kind="ExternalOutput")

    ENG = {"v": nc.vector, "a": nc.scalar}

    with tile.TileContext(nc) as tc:
        with (
            tc.tile_pool(name="fbc", bufs=2 * nql + 1) as p_fb,  # bf16 4-chunk
            tc.tile_pool(name="f8c", bufs=3) as p_f8,      # fp8 2-chunk
            tc.tile_pool(name="ft", bufs=2) as p_ft,       # f^T fp8 per batch
            tc.tile_pool(name="gsb", bufs=2 * nm) as p_g,
            tc.tile_pool(name="esb", bufs=nm) as p_e,
            tc.tile_pool(name="r8", bufs=4) as p_r8,       # fp8 R rows (paired)
            tc.tile_pool(name="stat", bufs=8 * nm) as p_stat,
            tc.tile_pool(name="outp", bufs=5) as p_out,
            tc.tile_pool(name="const", bufs=1) as p_const,
            tc.tile_pool(name="psg", bufs=1, space="PSUM") as p_psg,
            tc.tile_pool(name="pst", bufs=2, space="PSUM") as p_pst,
            tc.tile_pool(name="pso", bufs=2, space="PSUM") as p_pso,
        ):
            # --- constants (outside the timing loop) ---
            ident_rows = []
            for m in range(nm):
                t2 = p_const.tile([128, c], BF16, tag=f"id{m}",
                                  name=f"id{m}")
                nc.sync.dma_start(out=t2[:, :],
                                  in_=id_d[m * 128:(m + 1) * 128, :])
                ident_rows.append(t2)
            ident128 = ident_rows[0][:, 0:128]
            gam1 = p_const.tile([1, 1], F32, tag="gam1", name="gam1")
            nc.sync.dma_start(out=gam1[:, :], in_=gam_d[:, :])
            gamb = p_const.tile([128, 1], F32, tag="gamb", name="gamb")
            nc.gpsimd.partition_broadcast(gamb[:, :], gam1[:, :])
            # gamma * I (bf16) for the r8 build, and the 1 + gamma scalar
            igam = []
            for m in range(nm):
                t3 = p_const.tile([128, c], BF16, tag=f"ig{m}",
                                  name=f"ig{m}")
                nc.vector.tensor_scalar(
                    t3[:, :], ident_rows[m][:, :], gamb[:, :], None,
                    op0=AluOpType.mult)
                igam.append(t3)
            sgam = p_const.tile([128, 1], F32, tag="sgam", name="sgam")
            nc.vector.tensor_scalar(
                sgam[:, :], gamb[:, :], 1.0, None, op0=AluOpType.add)

            ctrs = {}

            def rot(site, cycle):
                i = ctrs.get(site, 0)
                ctrs[site] = i + 1
                return ENG[cycle[i % len(cycle)]]

            def rot_copy(site, cycle):
                eng = rot(site, cycle)
                return eng.copy if eng is nc.scalar else eng.tensor_copy

            def load4(b, ki, dt):
                """One SWDGE cast-DMA covering QL interleaved chunks."""
                base = b * n + ki * QL * 256
                src = x_d[base:base + QL * 256, :].rearrange(
                    "(q p j) c1 -> p q j c1", q=QL, p=128)
                t = (p_fb if dt == BF16 else p_f8).tile(
                    [128, QL, 2, c], dt, tag="fbc" if dt == BF16 else "f8c",
                    name=f"{'fb' if dt == BF16 else 'f8'}{b}_{ki}")
                nc.gpsimd.dma_start(out=t[:, :, :, :], in_=src)
                return t

            def gram_part(b, k, f8s, psg_rows):
                """Gram accumulation for chunk k (f8s: [128, 2, c] slice)."""
                for m in range(nm if ablate != "nogram" else 0):
                    lo = m * 128
                    nc.tensor.matmul(
                        psg_rows[m][:, 0:c - lo],
                        f8s[:, :, m * 128:(m + 1) * 128],
                        f8s[:, :, lo:c],
                        start=(k == 0),
                        stop=(k == nkc - 1),
                        perf_mode=mybir.MatmulPerfMode.DoubleRow,
                    )

            def tpose_part(b, k, fbs, ft8):
                """8 bf16 transposes per chunk; PSUM->SBUF copy casts fp8."""
                ps_t = p_pst.tile([128, 2, c], BF16, tag="pst",
                                  name=f"pst{b}_{k}")
                for j in range(2):
                    for m in range(nm):
                        nc.tensor.transpose(
                            ps_t[:, j, m * 128:(m + 1) * 128],
                            fbs[:, j, m * 128:(m + 1) * 128],
                            ident128,
                        )
                rot_copy("ftc", ftc_cycle)(ft8[:, k, :, :], ps_t[:, :, :])

            cast_state = {}

            def chunk_work(b, k, fb4, ft8, psg_rows):
                """Every even chunk, cast a 2-chunk slab to fp8 for the
                gram (interleaved with the stream so the in-order DVE/ACT
                queues alternate cast / ft-evac); transpose per chunk."""
                q = k % QL
                if k % 2 == 0:
                    f8t = p_f8.tile([128, 2, 2, c], FP8, tag="f8c",
                                    name=f"f8{b}_{k}")
                    rot_copy("cast", cast_cycle)(
                        f8t[:, :, :, :], fb4[:, q:q + 2, :, :])
                    cast_state[b] = f8t
                f8s = cast_state[b][:, k % 2, :, :]
                fbs = fb4[:, q, :, :]
                gram_part(b, k, f8s, psg_rows)
                tpose_part(b, k, fbs, ft8)

            def gram_finish(b, psg_rows):
                """PSUM G -> SBUF (bf16), mirror lower blocks, softmax ->
                R8 (fp8, gamma-scaled, zero diagonal, channel-pair packed)
                and the per-channel scale vector s_bc (bf16, broadcast)."""
                g_sb = []
                r8 = [p_r8.tile([128, 2, c], FP8, tag="r8",
                                name=f"r8_{b}_{t}") for t in range(2)]
                rec_st = p_stat.tile([128, nm], F32, tag="recst",
                                     name=f"recst{b}")

                def softmax_row(m):
                    t_g = g_sb[m]
                    nmax = p_stat.tile([128, 1], F32, tag="nmax",
                                       name=f"nmax{b}_{m}")
                    nc.vector.reduce_max(
                        nmax[:, :], t_g[:, :], axis=mybir.AxisListType.X,
                        negate=True,
                    )
                    e_sb = p_e.tile([128, c], BF16, tag="esb",
                                    name=f"e{b}_{m}")
                    esum = p_stat.tile([128, 1], F32, tag="esum",
                                       name=f"esum{b}_{m}")
                    nc.scalar.activation(
                        e_sb[:, :], t_g[:, :], AF.Exp,
                        bias=nmax[:, :], scale=1.0, accum_out=esum[:, :],
                    )
                    nc.vector.reciprocal(rec_st[:, m:m + 1], esum[:, :])
                    sc = p_stat.tile([128, 1], F32, tag="sc",
                                     name=f"sc{b}_{m}")
                    nc.vector.tensor_tensor(
                        sc[:, :], rec_st[:, m:m + 1], gamb[:, :],
                        op=AluOpType.mult,
                    )
                    # R8 row m: sc*E - gamma*I  (fp8).  Off-diagonal is
                    # gamma*A; the diagonal is gamma*(1/esum - 1), i.e. the
                    # per-channel correction to the scalar 1+gamma applied
                    # on the elementwise path (exactly 0 when softmax is
                    # saturated, since esum == 1).
                    nc.vector.scalar_tensor_tensor(
                        r8[m // 2][:, m % 2, :], e_sb[:, :], sc[:, :],
                        igam[m][:, :],
                        op0=AluOpType.mult, op1=AluOpType.subtract,
                    )

                for m in range(nm):
                    lo = m * 128
                    t_g = p_g.tile([128, c], BF16, tag="gsb", name=f"g{b}_{m}")
                    rot_copy("gcp", g_cycle)(t_g[:, lo:c],
                                             psg_rows[m][:, 0:c - lo])
                    g_sb.append(t_g)
                    if m:
                        tp = p_pst.tile([128, 2, c], BF16, tag="pst",
                                        name=f"gt{b}_{m}")
                        for d in range(m):
                            nc.tensor.transpose(
                                tp[:, 0, d * 128:(d + 1) * 128],
                                g_sb[d][:, m * 128:(m + 1) * 128],
                                ident128,
                            )
                        rot_copy("mir", mir_cycle)(
                            t_g[:, 0:m * 128], tp[:, 0, 0:m * 128])
                    softmax_row(m)

                return r8

            def make_fsk(b, ki, fb4):
                """fb4 <- fb4 * (1 + gamma) in place (per-partition scalar;
                ACT mul or DVE tensor_scalar, both 2-byte fast paths)."""
                for h in range(QL // 2):
                    eng = rot("fsk", fsk_cycle)
                    sl = fb4[:, 2 * h:2 * h + 2, :, :]
                    if eng is nc.scalar:
                        eng.mul(sl, sl, sgam[:, :])
                    else:
                        eng.tensor_scalar(sl, sl, sgam[:, :], None,
                                          op0=AluOpType.mult)

            def mm2_pair(b, kp, fb4s, ft8, r8):
                """MM2 + store for a pair of 256-row groups (k=2kp, 2kp+1):
                per-j fp8 DoubleRow matmuls -> f32 PSUM, ACT/DVE copy to
                m_sb (bf16), wide DVE add of the in-place fsk, one store."""
                m_sb = p_out.tile([128, QS, 2, c], BF16, tag="msb",
                                  name=f"m{b}_{kp}")
                for g in range(QS):
                    k = kp * QS + g
                    for j in range(2):
                        ps_o = p_pso.tile([128, c], F32, tag="pso",
                                          name=f"pso{b}_{k}_{j}")
                        for t in range(2):
                            nc.tensor.matmul(
                                ps_o[:, :],
                                ft8[:, k, j, t * 256:(t + 1) * 256]
                                    .rearrange("p (i q) -> p i q", i=2),
                                r8[t][:, :, :],
                                start=(t == 0),
                                stop=(t == 1),
                                perf_mode=mybir.MatmulPerfMode.DoubleRow,
                            )
                        rot_copy("mr", mr_cycle)(m_sb[:, g, j, :],
                                                 ps_o[:, :])
                # residual: m += f * (1 + gamma)  (f pre-scaled in place)
                nc.vector.tensor_tensor(
                    m_sb[:, :, :, :], m_sb[:, :, :, :], fb4s,
                    op=AluOpType.add,
                )
                base = b * n + kp * QS * 256
                nc.sync.dma_start(
                    out=y_d[base:base + QS * 256, :].rearrange(
                        "(q p j) c1 -> p q j c1", q=QS, p=128),
                    in_=m_sb[:, :, :, :],
                )

            def body(_iv=None):
                # ---- batch 0 stream ----
                ft8_0 = p_ft.tile([128, nkc, 2, c], FP8, tag="ft",
                                  name="ft0")
                psg0 = [p_psg.tile([128, c - m * 128], F32, tag=f"psg{m}",
                                   name=f"psg0_{m}") for m in range(nm)]
                fb0 = [load4(0, ki, BF16) for ki in range(nql)]
                if ablate == "loads":
                    for ki in range(nql):
                        f8k = p_f8.tile([128, QL, 2, c], FP8, tag="f8c",
                                        name=f"f8d0_{ki}")
                        nc.vector.tensor_copy(f8k[:, :, :, :],
                                              fb0[ki][:, :, :, :])
                    return
                fb1 = [load4(1, ki, BF16) for ki in range(nql)]
                for k in range(nkc):
                    chunk_work(0, k, fb0[k // QL], ft8_0, psg0)
                if ablate in ("nogram", "nofinish"):
                    return
                r8_0 = gram_finish(0, psg0)
                if ablate == "gram":
                    return
                for ki in range(nql):
                    make_fsk(0, ki, fb0[ki])

                # ---- batch 1 stream interleaved with batch 0 MM2 ----
                ft8_1 = p_ft.tile([128, nkc, 2, c], FP8, tag="ft",
                                  name="ft1")
                psg1 = [p_psg.tile([128, c - m * 128], F32, tag=f"psg{m}",
                                   name=f"psg1_{m}") for m in range(nm)]
                for kp in range(nkc // QS // 2):
                    for k in (4 * kp, 4 * kp + 1, 4 * kp + 2, 4 * kp + 3):
                        chunk_work(1, k, fb1[k // QL], ft8_1, psg1)
                    mm2_pair(0, kp,
                             fb0[(kp * QS) // QL]
                                [:, (kp * QS) % QL:(kp * QS) % QL + QS, :, :],
                             ft8_0, r8_0)
                if ablate == "phase4":
                    return
                r8_1 = gram_finish(1, psg1)
                for ki in range(nql):
                    make_fsk(1, ki, fb1[ki])
                for kp in range(nkc // QS // 2, nkc // QS):
                    mm2_pair(0, kp,
                             fb0[(kp * QS) // QL]
                                [:, (kp * QS) % QL:(kp * QS) % QL + QS, :, :],
                             ft8_0, r8_0)
                if ablate == "fin1":
                    return
                for kp in range(nkc // QS):
                    mm2_pair(1, kp,
                             fb1[(kp * QS) // QL]
                                [:, (kp * QS) % QL:(kp * QS) % QL + QS, :, :],
                             ft8_1, r8_1)

            if reps is None:
                body()
            else:
                assert reps % unroll == 0
                with tc.For_i(0, reps // unroll, 1,
                              staggered_reset=staggered,
                              hint_engines=(mybir.EngineType.PE,
                                            mybir.EngineType.DVE,
                                            mybir.EngineType.Activation,
                                            mybir.EngineType.Pool,
                                            mybir.EngineType.SP)) as iv:
                    for _ in range(unroll):
                        body(iv)

    nc.compile()
    return nc


_NC_CACHE = {}


def _get_nc():
    if "full" not in _NC_CACHE:
        _NC_CACHE["full"] = build_nc()
    return _NC_CACHE["full"]


def make_in_maps(inputs_np, gamma_np):
    """Shard full inputs into per-core in_maps."""
    x = np.ascontiguousarray(
        np.asarray(inputs_np, dtype=np.float32).reshape(B_FULL, N, C)
    )
    gam = np.asarray(gamma_np, dtype=np.float32).reshape(1, 1)
    ident = np.eye(C, dtype=np.float32).astype(ml_dtypes.bfloat16)
    in_maps = []
    for core in range(N_CORES):
        xs = x[core * B_LOC:(core + 1) * B_LOC].reshape(B_LOC * N, C)
        in_maps.append({
            "x": np.ascontiguousarray(xs),
            "gamma": gam,
            "ident": ident,
        })
    return in_maps


def kernel(inputs, gamma):
    nc = _get_nc()
    in_maps = make_in_maps(inputs, gamma)
    res = run_bass_kernel_spmd(nc, in_maps, core_ids=list(range(N_CORES)))
    outs = [np.asarray(res.results[c]["y"], dtype=np.float32)
            .reshape(B_LOC, N, C) for c in range(N_CORES)]
    y = np.concatenate(outs, axis=0).reshape(B_FULL, H, W, C)
    return y.astype(np.float32)


# revision 27
# speedup vs baseline: 1.3135x; 1.0282x over previous
"""Trainium2 Bass kernel for nn_CAM (channel attention module).

Reference (per batch b):
    f = x[b].reshape(N, C)                      # N = H*W = 4096, C = 512
    G = f^T f                                   # (C, C) channel gram
    A = softmax(G, axis=-1)
    out[b] = gamma * (f @ A) + x[b]

Algebraic folds:
  * out = f (.) s  +  f @ (gamma * R),  where s_d = 1 + gamma * A[d,d] and
    R = A - diag(A).  The gram's diagonal is ~N >> off-diag ~sqrt(N), so
    softmax concentrates on the diagonal; the dominant diagonal term is
    computed exactly on the elementwise path (bf16 f), and only the tiny
    off-diagonal remainder goes through the second matmul, which therefore
    runs in fp8 DoubleRow (2x bf16 PE throughput) with no accuracy cost.
    A[d,d] = 1/esum_d exactly, since exp(G_dd - rowmax) = exp(0) = 1.
  * symmetry: G == G^T, so only upper-triangular 128-blocks are computed
    (free dims 512/384/256/128); the 6 lower blocks are PE-transposed back.

Engine economy (the binding constraint once MM2 is fp8):
  * x is cast-LOADED TWICE by the SWDGE (fp32->bf16 and fp32->fp8), so no
    on-chip cast instructions exist at all; HBM has ~2.6x headroom.
  * Loads/stores are batched 4 chunks / 2 groups per DMA so the Pool/SP
    sequencers spend ~12us, not ~25us, issuing DMAs.
  * GPSIMD (Pool) executes NO tensor ops (hardware runs them far below the
    cost model's estimate); it only triggers DMAs + partition_broadcast.
  * fsk = f (.) s is computed in place over the bf16 chunk (all-SBUF
    all-bf16: DVE 2x/4x modes); the MM2 drain is an ACT/DVE plain PSUM
    copy and the residual add is a wide all-bf16 DVE tensor_tensor.

Layout: n rows are interleaved 2-per-partition (row 256k + 2p + j lives on
partition p, slice j of chunk k): 4KB load / 2KB store descriptors.  The
gram is invariant to the n-permutation; ft/MM2/store use it consistently.

Sharding: pure data-parallel over batch: 16 batches -> 8 cores x 2.
"""

import sys

if "/opt/trn_rl_repo" not in sys.path:
    sys.path.insert(0, "/opt/trn_rl_repo")

import numpy as np
import ml_dtypes

import concourse.bacc as bacc
import concourse.mybir as mybir
import concourse.tile as tile
from concourse.alu_op_type import AluOpType
from concourse.bass_utils import run_bass_kernel_spmd

F32 = mybir.dt.float32
BF16 = mybir.dt.bfloat16
FP8 = mybir.dt.float8e4
AF = mybir.ActivationFunctionType

N_CORES = 8
B_FULL, H, W, C = 16, 64, 64, 512
N = H * W                      # 4096 spatial positions per batch
B_LOC = B_FULL // N_CORES      # 2 batches per core
NM = C // 128                  # 4 channel blocks
NKC = N // 256                 # 16 interleaved 256-row chunks per batch
QL = 4                         # chunks per load DMA
QS = 2                         # groups per store DMA


def build_nc(b_loc=B_LOC, n=N, c=C, num_devices=N_CORES, reps=None,
             ablate=None, staggered=True, unroll=8,
             mr_cycle="aav", ftc_cycle="av", g_cycle="av", mir_cycle="a",
             fsk_cycle="av", cast_cycle="vva", dma_cast=False,
             **_legacy):
    """Build + compile the per-core Bass program.

    reps: if set, wrap the body in a hardware For_i loop (timing builds).
    *_cycle: per-site engine rotation strings (v=DVE, a=ACT).
    """
    nkc = n // 256   # interleaved 256-row chunks
    nm = c // 128
    nql = nkc // QL  # load DMAs per batch (per dtype)

    nc = bacc.Bacc(
        "TRN2",
        target_bir_lowering=False,
        debug=False,
        num_devices=num_devices,
    )

    x_d = nc.dram_tensor("x", [b_loc * n, c], F32, kind="ExternalInput")
    gam_d = nc.dram_tensor("gamma", [1, 1], F32, kind="ExternalInput")
    id_d = nc.dram_tensor("ident", [c, c], BF16, kind="ExternalInput")
    y_d = nc.dram_tensor("y", [b_loc * n, c], BF16, kind="ExternalOutput")

    ENG = {"v": nc.vector, "a": nc.scalar}

    with tile.TileContext(nc) as tc:
        with (
            tc.tile_pool(name="fbc", bufs=2 * nql + 1) as p_fb,  # bf16 4-chunk
            tc.tile_pool(name="f8c", bufs=3) as p_f8,      # fp8 2-chunk
            tc.tile_pool(name="ft", bufs=2) as p_ft,       # f^T fp8 per batch
            tc.tile_pool(name="gsb", bufs=2 * nm) as p_g,
            tc.tile_pool(name="esb", bufs=nm) as p_e,
            tc.tile_pool(name="r8", bufs=4) as p_r8,       # fp8 R rows (paired)
            tc.tile_pool(name="stat", bufs=8 * nm) as p_stat,
            tc.tile_pool(name="outp", bufs=5) as p_out,
            tc.tile_pool(name="const", bufs=1) as p_const,
            tc.tile_pool(name="psg", bufs=1, space="PSUM") as p_psg,
            tc.tile_pool(name="pst", bufs=2, space="PSUM") as p_pst,
            tc.tile_pool(name="pso", bufs=2, space="PSUM") as p_pso,
        ):
            # --- constants (outside the timing loop) ---
            ident_rows = []
            for m in range(nm):
                t2 = p_const.tile([128, c], BF16, tag=f"id{m}",
                                  name=f"id{m}")
                nc.sync.dma_start(out=t2[:, :],
                                  in_=id_d[m * 128:(m + 1) * 128, :])
                ident_rows.append(t2)
            ident128 = ident_rows[0][:, 0:128]
            gam1 = p_const.tile([1, 1], F32, tag="gam1", name="gam1")
            nc.sync.dma_start(out=gam1[:, :], in_=gam_d[:, :])
            gamb = p_const.tile([128, 1], F32, tag="gamb", name="gamb")
            nc.gpsimd.partition_broadcast(gamb[:, :], gam1[:, :])
            # gamma * I (bf16) for the r8 build, and the 1 + gamma scalar
            igam = []
            for m in range(nm):
                t3 = p_const.tile([128, c], BF16, tag=f"ig{m}",
                                  name=f"ig{m}")
                nc.vector.tensor_scalar(
                    t3[:, :], ident_rows[m][:, :], gamb[:, :], None,
                    op0=AluOpType.mult)
                igam.append(t3)
            sgam = p_const.tile([128, 1], F32, tag="sgam", name="sgam")
            nc.vector.tensor_scalar(
                sgam[:, :], gamb[:, :], 1.0, None, op0=AluOpType.add)

            ctrs = {}

            def rot(site, cycle):
                i = ctrs.get(site, 0)
                ctrs[site] = i + 1
                return ENG[cycle[i % len(cycle)]]

            def rot_copy(site, cycle):
                eng = rot(site, cycle)
                return eng.copy if eng is nc.scalar else eng.tensor_copy

            def load4(b, ki, dt):
                """One SWDGE cast-DMA covering QL interleaved chunks."""
                base = b * n + ki * QL * 256
                src = x_d[base:base + QL * 256, :].rearrange(
                    "(q p j) c1 -> p q j c1", q=QL, p=128)
                t = (p_fb if dt == BF16 else p_f8).tile(
                    [128, QL, 2, c], dt, tag="fbc" if dt == BF16 else "f8c",
                    name=f"{'fb' if dt == BF16 else 'f8'}{b}_{ki}")
                nc.gpsimd.dma_start(out=t[:, :, :, :], in_=src)
                return t

            def gram_part(b, k, f8s, psg_rows):
                """Gram accumulation for chunk k (f8s: [128, 2, c] slice)."""
                for m in range(nm if ablate != "nogram" else 0):
                    lo = m * 128
                    nc.tensor.matmul(
                        psg_rows[m][:, 0:c - lo],
                        f8s[:, :, m * 128:(m + 1) * 128],
                        f8s[:, :, lo:c],
                        start=(k == 0),
                        stop=(k == nkc - 1),
                        perf_mode=mybir.MatmulPerfMode.DoubleRow,
                    )

            def tpose_part(b, k, fbs, ft8):
                """8 bf16 transposes per chunk; PSUM->SBUF copy casts fp8."""
                ps_t = p_pst.tile([128, 2, c], BF16, tag="pst",
                                  name=f"pst{b}_{k}")
                for j in range(2):
                    for m in range(nm):
                        nc.tensor.transpose(
                            ps_t[:, j, m * 128:(m + 1) * 128],
                            fbs[:, j, m * 128:(m + 1) * 128],
                            ident128,
                        )
                rot_copy("ftc", ftc_cycle)(ft8[:, k, :, :], ps_t[:, :, :])

            cast_state = {}

            def chunk_work(b, k, fb4, ft8, psg_rows):
                """Every even chunk, cast a 2-chunk slab to fp8 for the
                gram (interleaved with the stream so the in-order DVE/ACT
                queues alternate cast / ft-evac); transpose per chunk."""
                q = k % QL
                if k % 2 == 0:
                    f8t = p_f8.tile([128, 2, 2, c], FP8, tag="f8c",
                                    name=f"f8{b}_{k}")
                    rot_copy("cast", cast_cycle)(
                        f8t[:, :, :, :], fb4[:, q:q + 2, :, :])
                    cast_state[b] = f8t
                f8s = cast_state[b][:, k % 2, :, :]
                fbs = fb4[:, q, :, :]
                gram_part(b, k, f8s, psg_rows)
                tpose_part(b, k, fbs, ft8)

            def gram_finish(b, psg_rows):
                """PSUM G -> SBUF (bf16), mirror lower blocks, softmax ->
                R8 (fp8, gamma-scaled, zero diagonal, channel-pair packed)
                and the per-channel scale vector s_bc (bf16, broadcast)."""
                g_sb = []
                r8 = [p_r8.tile([128, 2, c], FP8, tag="r8",
                                name=f"r8_{b}_{t}") for t in range(2)]
                rec_st = p_stat.tile([128, nm], F32, tag="recst",
                                     name=f"recst{b}")

                def softmax_row(m):
                    t_g = g_sb[m]
                    nmax = p_stat.tile([128, 1], F32, tag="nmax",
                                       name=f"nmax{b}_{m}")
                    nc.vector.reduce_max(
                        nmax[:, :], t_g[:, :], axis=mybir.AxisListType.X,
                        negate=True,
                    )
                    e_sb = p_e.tile([128, c], BF16, tag="esb",
                                    name=f"e{b}_{m}")
                    esum = p_stat.tile([128, 1], F32, tag="esum",
                                       name=f"esum{b}_{m}")
                    nc.scalar.activation(
                        e_sb[:, :], t_g[:, :], AF.Exp,
                        bias=nmax[:, :], scale=1.0, accum_out=esum[:, :],
                    )
                    nc.vector.reciprocal(rec_st[:, m:m + 1], esum[:, :])
                    sc = p_stat.tile([128, 1], F32, tag="sc",
                                     name=f"sc{b}_{m}")
                    nc.vector.tensor_tensor(
                        sc[:, :], rec_st[:, m:m + 1], gamb[:, :],
                        op=AluOpType.mult,
                    )
                    # R8 row m: sc*E - gamma*I  (fp8).  Off-diagonal is
                    # gamma*A; the diagonal is gamma*(1/esum - 1), i.e. the
                    # per-channel correction to the scalar 1+gamma applied
                    # on the elementwise path (exactly 0 when softmax is
                    # saturated, since esum == 1).
                    nc.vector.scalar_tensor_tensor(
                        r8[m // 2][:, m % 2, :], e_sb[:, :], sc[:, :],
                        igam[m][:, :],
                        op0=AluOpType.mult, op1=AluOpType.subtract,
                    )

                for m in range(nm):
                    lo = m * 128
                    t_g = p_g.tile([128, c], BF16, tag="gsb", name=f"g{b}_{m}")
                    rot_copy("gcp", g_cycle)(t_g[:, lo:c],
                                             psg_rows[m][:, 0:c - lo])
                    g_sb.append(t_g)
                    if m:
                        tp = p_pst.tile([128, 2, c], BF16, tag="pst",
                                        name=f"gt{b}_{m}")
                        for d in range(m):
                            nc.tensor.transpose(
                                tp[:, 0, d * 128:(d + 1) * 128],
                                g_sb[d][:, m * 128:(m + 1) * 128],
                                ident128,
                            )
                        rot_copy("mir", mir_cycle)(
                            t_g[:, 0:m * 128], tp[:, 0, 0:m * 128])
                    softmax_row(m)

                return r8

            def make_fsk(b, ki, fb4):
                """fb4 <- fb4 * (1 + gamma) in place (per-partition scalar;
                ACT mul or DVE tensor_scalar, both 2-byte fast paths)."""
                for h in range(QL // 2):
                    eng = rot("fsk", fsk_cycle)
                    sl = fb4[:, 2 * h:2 * h + 2, :, :]
                    if eng is nc.scalar:
                        eng.mul(sl, sl, sgam[:, :])
                    else:
                        eng.tensor_scalar(sl, sl, sgam[:, :], None,
                                          op0=AluOpType.mult)

            def mm2_pair(b, kp, fb4s, ft8, r8):
                """MM2 + store for a pair of 256-row groups (k=2kp, 2kp+1):
                per-j fp8 DoubleRow matmuls -> f32 PSUM, ACT/DVE copy to
                m_sb (bf16), wide DVE add of the in-place fsk, one store."""
                m_sb = p_out.tile([128, QS, 2, c], BF16, tag="msb",
                                  name=f"m{b}_{kp}")
                for g in range(QS):
                    k = kp * QS + g
                    for j in range(2):
                        ps_o = p_pso.tile([128, c], F32, tag="pso",
                                          name=f"pso{b}_{k}_{j}")
                        for t in range(2):
                            nc.tensor.matmul(
                                ps_o[:, :],
                                ft8[:, k, j, t * 256:(t + 1) * 256]
                                    .rearrange("p (i q) -> p i q", i=2),
                                r8[t][:, :, :],
                                start=(t == 0),
                                stop=(t == 1),
                                perf_mode=mybir.MatmulPerfMode.DoubleRow,
                            )
                        rot_copy("mr", mr_cycle)(m_sb[:, g, j, :],
                                                 ps_o[:, :])
                # residual: m += f * (1 + gamma)  (f pre-scaled in place)
                nc.vector.tensor_tensor(
                    m_sb[:, :, :, :], m_sb[:, :, :, :], fb4s,
                    op=AluOpType.add,
                )
                base = b * n + kp * QS * 256
                nc.sync.dma_start(
                    out=y_d[base:base + QS * 256, :].rearrange(
                        "(q p j) c1 -> p q j c1", q=QS, p=128),
                    in_=m_sb[:, :, :, :],
                )

            def body(_iv=None):
                # ---- batch 0 stream ----
                ft8_0 = p_ft.tile([128, nkc, 2, c], FP8, tag="ft",
                                  name="ft0")
                psg0 = [p_psg.tile([128, c - m * 128], F32, tag=f"psg{m}",
                                   name=f"psg0_{m}") for m in range(nm)]
                fb0 = [load4(0, ki, BF16) for ki in range(nql)]
                if ablate == "loads":
                    for ki in range(nql):
                        f8k = p_f8.tile([128, QL, 2, c], FP8, tag="f8c",
                                        name=f"f8d0_{ki}")
                        nc.vector.tensor_copy(f8k[:, :, :, :],
                                              fb0[ki][:, :, :, :])
                    return
                fb1 = [load4(1, ki, BF16) for ki in range(nql)]
                for k in range(nkc):
                    chunk_work(0, k, fb0[k // QL], ft8_0, psg0)
                if ablate in ("nogram", "nofinish"):
                    return
                r8_0 = gram_finish(0, psg0)
                if ablate == "gram":
                    return
                for ki in range(nql):
                    make_fsk(0, ki, fb0[ki])

                # ---- batch 1 stream interleaved with batch 0 MM2 ----
                ft8_1 = p_ft.tile([128, nkc, 2, c], FP8, tag="ft",
                                  name="ft1")
                psg1 = [p_psg.tile([128, c - m * 128], F32, tag=f"psg{m}",
                                   name=f"psg1_{m}") for m in range(nm)]
                for kp in range(nkc // QS // 2):
                    for k in (4 * kp, 4 * kp + 1, 4 * kp + 2, 4 * kp + 3):
                        chunk_work(1, k, fb1[k // QL], ft8_1, psg1)
                    mm2_pair(0, kp,
                             fb0[(kp * QS) // QL]
                                [:, (kp * QS) % QL:(kp * QS) % QL + QS, :, :],
                             ft8_0, r8_0)
                if ablate == "phase4":
                    return
                r8_1 = gram_finish(1, psg1)
                for ki in range(nql):
                    make_fsk(1, ki, fb1[ki])
                for kp in range(nkc // QS // 2, nkc // QS):
                    mm2_pair(0, kp,
                             fb0[(kp * QS) // QL]
                                [:, (kp * QS) % QL:(kp * QS) % QL + QS, :, :],
                             ft8_0, r8_0)
                if ablate == "fin1":
                    return
                for kp in range(nkc // QS):
                    mm2_pair(1, kp,
                             fb1[(kp * QS) // QL]
                                [:, (kp * QS) % QL:(kp * QS) % QL + QS, :, :],
                             ft8_1, r8_1)

            if reps is None:
                body()
            else:
                assert reps % unroll == 0
                with tc.For_i(0, reps // unroll, 1,
                              staggered_reset=staggered,
                              hint_engines=(mybir.EngineType.PE,
                                            mybir.EngineType.DVE,
                                            mybir.EngineType.Activation,
                                            mybir.EngineType.Pool,
                                            mybir.EngineType.SP)) as iv:
                    for _ in range(unroll):
                        body(iv)

    nc.compile()
    return nc


_NC_CACHE = {}


def _get_nc():
    if "full" not in _NC_CACHE:
        _NC_CACHE["full"] = build_nc()
    return _NC_CACHE["full"]


def make_in_maps(inputs_np, gamma_np):
    """Shard full inputs into per-core in_maps."""
    x = np.ascontiguousarray(
        np.asarray(inputs_np, dtype=np.float32).reshape(B_FULL, N, C)
    )
    gam = np.asarray(gamma_np, dtype=np.float32).reshape(1, 1)
    ident = np.eye(C, dtype=np.float32).astype(ml_dtypes.bfloat16)
    in_maps = []
    for core in range(N_CORES):
        xs = x[core * B_LOC:(core + 1) * B_LOC].reshape(B_LOC * N, C)
        in_maps.append({
            "x": np.ascontiguousarray(xs),
            "gamma": gam,
            "ident": ident,
        })
    return in_maps


def kernel(inputs, gamma):
    nc = _get_nc()
    in_maps = make_in_maps(inputs, gamma)
    res = run_bass_kernel_spmd(nc, in_maps, core_ids=list(range(N_CORES)))
    outs = [np.asarray(res.results[c]["y"], dtype=np.float32)
            .reshape(B_LOC, N, C) for c in range(N_CORES)]
    y = np.concatenate(outs, axis=0).reshape(B_FULL, H, W, C)
    return y.astype(np.float32)
